# revision 37
# baseline (speedup 1.0000x reference)
"""Trainium2 Bass kernel for a full-attention MHA layer (B=2, S=2048, HID=2048,
16 heads, head_dim=128, RoPE, no mask), sharded over 8 NeuronCores as
2 batches x 4 head-groups (4 heads per core).

Per-core dataflow (feature-major, so no probability transposes are needed):
  hidT     = host-pretransposed hidden              [k,t]  (bf16, DMA'd direct)
  qT,kT    = w_qkvT.T @ hidT                        [d,t]  per head + RoPE
  v        = hidT.T @ w_vT                          [t,d]  natural layout
  ST       = kT.T @ qT                              [tk,tq] scores transposed
  PT       = exp(ST * scale)                        (ACT, PSUM->SBUF fused)
  den      = onesT.T @ PT                           partition sum, replicated
  OT       = (v.T @ PT) * (1/den)                   [d,tq]
  OUT      = OT.T @ w_oT                            [t,o]  partial over heads,
                                                    stored bf16
Host sums the 4 per-batch partial OUTs in fp32.
"""
import numpy as np
import ml_dtypes

import concourse.bass as bass
import concourse.mybir as mybir
from concourse import bacc, tile

B, S, HID = 2, 2048, 2048
NH, HD = 16, 128
G = 4                 # head-groups = cores per batch
NHL = NH // G         # heads per core
KO = HID // 128       # 16 contraction chunks
TS = 512              # token slice for the projection phase
NSL = S // TS         # 4
TQ = 512              # query-tile width in attention
NTQ = S // TQ         # 4
NTK = S // 128        # 16 key chunks
FQK = NHL * HD        # 512 features for q (and k) per core
FV = NHL * HD         # 512 features for v per core
BF16 = mybir.dt.bfloat16
F32 = mybir.dt.float32
SCALE = 1.0 / float(np.sqrt(HD))

N_CORES = 8


O_HID = 0
O_WQK = O_HID + NSL * KO * TS          # 32768
O_WV = O_WQK + NHL * KO * 2 * HD       # 49152
O_WO = O_WV + KO * FV                  # 57344
O_COS = O_WO + NHL * HID               # 65536
O_SSIN = O_COS + S                     # 67584
N_IN = O_SSIN + S                      # 69632


def _emit(nc, tc, inp, outp, repeats=1):
    from contextlib import ExitStack
    ctx = ExitStack()
    with ctx:
        const = ctx.enter_context(tc.tile_pool(name="const", bufs=1))
        persist = ctx.enter_context(tc.tile_pool(name="persist", bufs=1))
        work = ctx.enter_context(tc.tile_pool(name="work", bufs=2))
        small = ctx.enter_context(tc.tile_pool(name="small", bufs=2))
        psA = ctx.enter_context(tc.tile_pool(name="psA", bufs=4, space="PSUM"))
        # psA 'mm' tiles are [128, 1024] (2 banks) in the attention phase and
        # [128, 512] (1 bank) in the projection phase; bufs is set per-tag.
        psB = ctx.enter_context(tc.tile_pool(name="psB", bufs=2, space="PSUM"))
        psC = ctx.enter_context(tc.tile_pool(name="psC", bufs=2, space="PSUM"))

        # ---- constants, loaded in first-use order on the gpsimd queue ----
        # (all non-activation inputs live in one packed [128, N_IN] bf16
        # tensor; slices below are per-partition element offsets)
        ones_sb = const.tile([128, 128], BF16)
        nc.vector.memset(ones_sb, 1.0)
        wqk_sb = const.tile([128, NHL, KO, 2 * HD], BF16)
        # first f-tile-pair group arrives in 4-ko subchunks so the first
        # projection matmul can start as soon as ko 0-3 land
        for kq in range(4):
            nc.gpsimd.dma_start(wqk_sb[:, 0, 4 * kq:4 * (kq + 1)],
                                inp[:, O_WQK + 1024 * kq:O_WQK + 1024 * (kq + 1)])
        for ftp in range(1, NHL):
            nc.gpsimd.dma_start(wqk_sb[:, ftp],
                                inp[:, O_WQK + 4096 * ftp:O_WQK + 4096 * (ftp + 1)])
        wv_sb = const.tile([128, KO, FV], BF16)
        nc.gpsimd.dma_start(wv_sb, inp[:, O_WV:O_WV + KO * FV])
        cos_sb = const.tile([128, S], BF16)
        nc.gpsimd.dma_start(cos_sb, inp[:, O_COS:O_COS + S])
        ssin_sb = const.tile([128, S], BF16)
        nc.gpsimd.dma_start(ssin_sb, inp[:, O_SSIN:O_SSIN + S])
        wo_sb = const.tile([128, NHL, HID], BF16)
        nc.gpsimd.dma_start(wo_sb, inp[:, O_WO:O_WO + NHL * HID])

        for _rep in range(repeats):
            _emit_body(nc, tc, inp, outp, wqk_sb, wv_sb, wo_sb, cos_sb, ssin_sb,
                       ones_sb, persist, work, small, psA, psB, psC)


def _emit_body(nc, tc, inp, outp, wqk_sb, wv_sb, wo_sb, cos_sb, ssin_sb,
               ones_sb, persist, work, small, psA, psB, psC):
        # ---- persistent activations ----
        qT = persist.tile([128, NHL, S], BF16, tag="qT", bufs=1)   # [d, h, t]
        kT = persist.tile([128, NHL, S], BF16, tag="kT", bufs=1)   # [d, h, t]
        vN = persist.tile([128, NTK, FV], BF16, tag="vN", bufs=1)  # [t%128, t//128, f]
        oT = persist.tile([128, NHL, S], BF16, tag="oT", bufs=1)   # [d, h, tq]

        # ======== attention score pipeline (used by phases below) ========
        # Flat (tq, h, pair) score pipeline rolling ~AHEAD pairs ahead of the
        # PV consumer, across iteration boundaries, so the exp latency never
        # stalls the PE.  Scores are computed two key-chunks at a time into a
        # two-bank PSUM tile so each ACT exp instruction covers 1024 elements
        # (halves ACT instruction count; ACT is the attention-phase
        # co-bottleneck).  Den matmuls are emitted at iteration end from bf16
        # pair-sums built on DVE as the prob chunks complete.
        from collections import deque
        seq = [(tqi, h) for tqi in range(NTQ) for h in range(NHL)]
        PAIRS = NTK // 2
        AHEAD = 3

        def emit_score_pair(it_idx, pr):
            tqi, h = seq[it_idx]
            tq0 = tqi * TQ
            ps = psA.tile([128, 2 * TQ], F32, tag="mm", bufs=2, name="ps")
            for half in range(2):
                tkc = 2 * pr + half
                nc.tensor.matmul(ps[:, half * TQ:(half + 1) * TQ],
                                 kT[:, h, tkc * 128:(tkc + 1) * 128],
                                 qT[:, h, tq0:tq0 + TQ],
                                 start=True, stop=True)
            pt = small.tile([128, 2 * TQ], BF16, tag="pt", bufs=6, name="pt")
            nc.scalar.activation(pt, ps,
                                 mybir.ActivationFunctionType.Exp,
                                 scale=SCALE)
            return pt

        stream = [(it, p) for it in range(len(seq)) for p in range(PAIRS)]
        pending = deque()
        ptr = 0

        def fill_to(pos):
            nonlocal ptr
            while ptr < len(stream) and ptr <= pos + AHEAD:
                pending.append(emit_score_pair(*stream[ptr]))
                ptr += 1

        # ---- PE warmup: dummy matmuls with no DMA dependency so the HAM
        # clock-gate and p-state ramp complete during the first weight loads
        warm = psB.tile([128, TQ], F32, tag="acc")
        for _ in range(16):
            nc.tensor.matmul(warm[:, :128], ones_sb, ones_sb, start=True, stop=True)

        # ======== Phase A+B: hidT load + QKV projections + RoPE ========
        for ts_i in range(NSL):
            t0 = ts_i * TS
            hT = work.tile([128, KO, TS], BF16, tag="hT", name="hT")
            for ko in range(KO):
                base = O_HID + ts_i * KO * TS + ko * TS
                nc.sync.dma_start(hT[:, ko, :], inp[:, base:base + TS])
            # q,k projections: f-tile = one head's 128 dims (0-3 q, 4-7 k);
            # two f-tiles share one two-bank PSUM tile.  ko-major emission so
            # PE consumes the hT/wqk DMA streams in arrival order.
            for ftp in range(NHL):
                ps = psA.tile([128, 2 * TS], F32, tag="mm", bufs=2)
                for ko in range(KO):
                    for half in range(2):
                        nc.tensor.matmul(ps[:, half * TS:(half + 1) * TS],
                                         wqk_sb[:, ftp, ko, half * HD:(half + 1) * HD],
                                         hT[:, ko, :],
                                         start=(ko == 0), stop=(ko == KO - 1))
                for half in range(2):
                    ft = 2 * ftp + half
                    psh = ps[:, half * TS:(half + 1) * TS]
                    if ft < NHL:
                        dest = qT[:, ft, t0:t0 + TS]
                        dlo = qT[0:64, ft, t0:t0 + TS]
                        dhi = qT[64:128, ft, t0:t0 + TS]
                    else:
                        dest = kT[:, ft - NHL, t0:t0 + TS]
                        dlo = kT[0:64, ft - NHL, t0:t0 + TS]
                        dhi = kT[64:128, ft - NHL, t0:t0 + TS]
                    nc.vector.tensor_copy(dest, psh)
                    # RoPE: dest = dest*cos + swap(dest)*ssin (sign in ssin)
                    sw = small.tile([128, TS], BF16, tag="sw", bufs=2)
                    nc.gpsimd.dma_start(sw[0:64, :], dhi)
                    nc.gpsimd.dma_start(sw[64:128, :], dlo)
                    nc.vector.tensor_mul(dest, dest, cos_sb[:, t0:t0 + TS])
                    nc.vector.tensor_mul(sw, sw, ssin_sb[:, t0:t0 + TS])
                    nc.vector.tensor_add(dest, dest, sw)
            # prime the attention score pipeline before the last slice's
            # v-projection so the first exps overlap the v matmuls
            if ts_i == NSL - 1:
                fill_to(-1)
            # v projection in natural [t, f] layout; two t-tiles per PSUM tile
            for ttp in range(TS // 256):
                ps = psA.tile([128, 2 * TS], F32, tag="mm", bufs=2)
                for ko in range(KO):
                    for half in range(2):
                        tt = 2 * ttp + half
                        nc.tensor.matmul(ps[:, half * TS:(half + 1) * TS],
                                         hT[:, ko, tt * 128:(tt + 1) * 128],
                                         wv_sb[:, ko, :],
                                         start=(ko == 0), stop=(ko == KO - 1))
                for half in range(2):
                    tt = 2 * ttp + half
                    nc.vector.tensor_copy(vN[:, ts_i * (TS // 128) + tt, :],
                                          ps[:, half * TS:(half + 1) * TS])

        # ======== Phase C+D: attention, with out-proj interleaved per tq ========
        for it_idx, (tqi, h) in enumerate(seq):
            tq0 = tqi * TQ
            pv = psB.tile([128, TQ], F32, tag="acc")
            pvc = small.tile([128, TQ], BF16, tag="pvc", bufs=2, name="pvc")
            gsums = []
            prev_pt = None
            last_pts = None
            for p in range(PAIRS):
                fill_to(it_idx * PAIRS + p)
                pt = pending.popleft()
                for half in range(2):
                    c = 2 * p + half
                    nc.tensor.matmul(pv,
                                     vN[:, c, h * HD:(h + 1) * HD],
                                     pt[:, half * TQ:(half + 1) * TQ],
                                     start=(c == 0), stop=(c == NTK - 1))
                if p % 2 == 0:
                    prev_pt = pt
                elif p < PAIRS - 1:
                    s1 = small.tile([128, TQ], BF16, tag="ptsum", bufs=6, name="s1")
                    nc.vector.tensor_add(s1, prev_pt[:, :TQ], prev_pt[:, TQ:])
                    s2 = small.tile([128, TQ], BF16, tag="pts2", bufs=2, name="s2")
                    nc.vector.tensor_add(s2, pt[:, :TQ], pt[:, TQ:])
                    nc.vector.tensor_add(s1, s1, s2)
                    gsums.append(s1)
                    prev_pt = None
                else:
                    last_pts = (prev_pt, pt)
            last_it = it_idx == len(seq) - 1
            if not last_it:
                # evacuate pv to SBUF first (DVE) so the next iteration's
                # first PV matmul gets its PSUM bank back quickly
                nc.vector.tensor_copy(pvc, pv)
            prev_pt, pt = last_pts
            s1 = small.tile([128, TQ], BF16, tag="ptsum", bufs=6, name="s1")
            nc.vector.tensor_add(s1, prev_pt[:, :TQ], prev_pt[:, TQ:])
            s2 = small.tile([128, TQ], BF16, tag="pts2", bufs=2, name="s2")
            nc.vector.tensor_add(s2, pt[:, :TQ], pt[:, TQ:])
            nc.vector.tensor_add(s1, s1, s2)
            gsums.append(s1)
            # merge the 4 group sums to two tiles on DVE so den costs two
            # PE matmuls (which also cover the pvc copy latency)
            g1 = small.tile([128, TQ], BF16, tag="ptsum", bufs=6, name="g1")
            nc.vector.tensor_add(g1, gsums[0], gsums[1])
            g2 = small.tile([128, TQ], BF16, tag="pts2", bufs=2, name="g2")
            nc.vector.tensor_add(g2, gsums[2], gsums[3])
            den = psB.tile([128, TQ], F32, tag="acc")
            nc.tensor.matmul(den, ones_sb, g1, start=True, stop=False)
            nc.tensor.matmul(den, ones_sb, g2, start=False, stop=True)
            rec = small.tile([128, TQ], F32, tag="rec", bufs=2)
            nc.vector.reciprocal(rec, den)
            if not last_it:
                nc.vector.tensor_mul(oT[:, h, tq0:tq0 + TQ], pvc, rec)
            else:
                # final iteration: normalize straight from PSUM in ascending
                # 128-token chunks so the out-projection's h3 matmuls
                # unblock progressively
                for j in range(TQ // 128):
                    c0, c1 = j * 128, (j + 1) * 128
                    nc.vector.tensor_mul(oT[:, h, tq0 + c0:tq0 + c1],
                                         pv[:, c0:c1], rec[:, c0:c1])
            if h != NHL - 1:
                continue
            # out-projection for the t-range covered by this tq tile
            for tt in range(tqi * (TQ // 128), (tqi + 1) * (TQ // 128)):
                for ot in range(HID // 512):
                    ps = psC.tile([128, 512], F32, tag="out")
                    for h in range(NHL):
                        nc.tensor.matmul(ps,
                                         oT[:, h, tt * 128:(tt + 1) * 128],
                                         wo_sb[:, h, ot * 512:(ot + 1) * 512],
                                         start=(h == 0), stop=(h == NHL - 1))
                    ob = small.tile([128, 512], BF16, tag="ob", bufs=3)
                    if (tt + ot) % 2 == 0:
                        nc.vector.tensor_copy(ob, ps)
                    else:
                        nc.scalar.copy(ob, ps)
                    nc.sync.dma_start(outp[tt * 128:(tt + 1) * 128, ot * 512:(ot + 1) * 512], ob)


def build(repeats=1):
    nc = bacc.Bacc("TRN2", target_bir_lowering=False, debug=False)
    inp = nc.dram_tensor("inp", [128, N_IN], BF16, kind="ExternalInput")
    outp = nc.dram_tensor("outp", [S, HID], BF16, kind="ExternalOutput")
    with tile.TileContext(nc) as tc:
        _emit(nc, tc, inp.ap(), outp.ap(), repeats=repeats)
    nc.compile()
    return nc


def shard_inputs(hidden_states, cos, sin, w_qkv, w_o):
    """Build the 8 per-core input maps (host-side layout prep)."""
    hidden_states = np.asarray(hidden_states, dtype=np.float32)
    cos = np.asarray(cos, dtype=np.float32)
    sin = np.asarray(sin, dtype=np.float32)
    w_qkv = np.asarray(w_qkv, dtype=np.float32)
    w_o = np.asarray(w_o, dtype=np.float32)

    cosT = np.ascontiguousarray(cos[:, 0, :].T).astype(ml_dtypes.bfloat16)
    sT = sin[:, 0, :].T.copy()
    sT[:64] = -sT[:64]
    ssinT = np.ascontiguousarray(sT).astype(ml_dtypes.bfloat16)

    # hidT[p, si, ko, t] = hidden[b][si*TS + t, ko*128 + p]
    hidT_b = []
    for b in range(B):
        ht = hidden_states[b].T.reshape(KO, 128, NSL, TS).transpose(1, 2, 0, 3)
        hidT_b.append(np.ascontiguousarray(ht).astype(ml_dtypes.bfloat16))

    woT = w_o.T  # [j, o]
    in_maps = []
    for c in range(N_CORES):
        b, g = divmod(c, G)
        rows = np.concatenate([
            w_qkv[FQK * g: FQK * (g + 1)],
            w_qkv[NH * HD + FQK * g: NH * HD + FQK * (g + 1)],
        ], axis=0)                                   # [1024, 2048] q then k
        # wqk[p, ftp, ko, half*HD + j] = rows[(2*ftp+half)*HD + j, ko*128 + p]
        wqk_pack = np.ascontiguousarray(
            rows.reshape(NHL, 2 * HD, KO, 128).transpose(3, 0, 2, 1)
        ).astype(ml_dtypes.bfloat16)                 # [128, NHL, KO, 256]
        rv = w_qkv[2 * NH * HD + FQK * g: 2 * NH * HD + FQK * (g + 1)]  # [512, 2048]
        wv_pack = np.ascontiguousarray(
            rv.reshape(FV, KO, 128).transpose(2, 1, 0)
        ).astype(ml_dtypes.bfloat16)                 # [128, KO, 512]
        wo_pack = np.ascontiguousarray(
            woT[FQK * g: FQK * (g + 1)].reshape(NHL, 128, HID).transpose(1, 0, 2)
        ).astype(ml_dtypes.bfloat16)                 # [128, NHL, 2048]
        in_maps.append({
            "inp": np.ascontiguousarray(np.concatenate([
                hidT_b[b].reshape(128, -1),
                wqk_pack.reshape(128, -1),
                wv_pack.reshape(128, -1),
                wo_pack.reshape(128, -1),
                cosT, ssinT], axis=1)),
        })
    return in_maps


def gather_outputs(results):
    """results: list of 8 dicts with 'outp' -> full [B, S, HID] output."""
    out = np.zeros((B, S, HID), dtype=np.float32)
    for c in range(N_CORES):
        b = c // G
        out[b] += results[c]["outp"].astype(np.float32)
    return out


# ---------------- cached runner over PJRT/axon ----------------
_RUNNER = None


def _make_runner(repeats=1):
    import jax
    from jax.sharding import Mesh, PartitionSpec, NamedSharding
    from jax.experimental.shard_map import shard_map
    from concourse import bass2jax

    nc = build(repeats=repeats)
    bass2jax.install_neuronx_cc_hook()
    partition_name = nc.partition_id_tensor.name if nc.partition_id_tensor else None
    in_names, out_names, out_avals = [], [], []
    for alloc in nc.m.functions[0].allocations:
        if not isinstance(alloc, mybir.MemoryLocationSet):
            continue
        name = alloc.memorylocations[0].name
        if alloc.kind == "ExternalInput":
            if name != partition_name:
                in_names.append(name)
        elif alloc.kind == "ExternalOutput":
            out_names.append(name)
            out_avals.append(jax.core.ShapedArray(
                tuple(alloc.tensor_shape), mybir.dt.np(alloc.dtype)))
    n_params = len(in_names)
    all_in_names = list(in_names) + list(out_names)
    if partition_name is not None:
        all_in_names.append(partition_name)

    import hashlib
    import os as _os
    _tag = hashlib.sha256(open(__file__, "rb").read()
                          + str(repeats).encode()
                          + _os.environ.get("BASS_KERNEL_TAG", "").encode()).hexdigest()[:12]

    def _body(*args):
        operands = list(args)
        if partition_name is not None:
            operands.append(bass2jax.partition_id_tensor())
        outs = bass2jax._bass_exec_p.bind(
            *operands,
            out_avals=tuple(out_avals),
            in_names=tuple(all_in_names),
            out_names=tuple(out_names),
            lowering_input_output_aliases=(),
            sim_require_finite=True,
            sim_require_nnan=True,
            nc=nc,
        )
        return tuple(outs)

    devices = jax.devices()[:N_CORES]
    mesh = Mesh(np.asarray(devices), ("core",))
    n_outs = len(out_names)
    in_specs = (PartitionSpec("core"),) * (n_params + n_outs)
    out_specs = (PartitionSpec("core"),) * n_outs
    donate = tuple(range(n_params, n_params + n_outs))
    _body.__name__ = f"body_{_tag}"
    _sharded = shard_map(_body, mesh=mesh, in_specs=in_specs, out_specs=out_specs,
                         check_rep=False)

    def _entry(*args):
        return _sharded(*args)
    _entry.__name__ = f"bass_attn_{_tag}"
    fn = jax.jit(_entry, donate_argnums=donate, keep_unused=True)
    sharding = NamedSharding(mesh, PartitionSpec("core"))

    class Runner:
        def __init__(self):
            self.fn = fn
            self.nc = nc
            self.entry = _entry
            self.in_names = in_names
            self.out_names = out_names
            self.out_avals = out_avals
            self.sharding = sharding
            self._fast = None

        def fast(self, dev_in, outs):
            """C++ fast-path dispatch (bass_effect suppressed)."""
            if self._fast is None:
                import jax
                from concourse.bass2jax import fast_dispatch_compile
                entry = self.entry
                n_outs = len(self.out_names)
                donate2 = tuple(range(len(self.in_names),
                                      len(self.in_names) + n_outs))

                def _compile():
                    f = jax.jit(entry, donate_argnums=donate2, keep_unused=True)
                    return f.lower(*dev_in, *outs).compile()
                self._fast = fast_dispatch_compile(_compile)
            return self._fast

        def make_chain(self, k):
            """jit fn applying the kernel k times, outputs threaded through."""
            import jax
            n_params = len(self.in_names)
            entry = self.entry

            def entry_k(*args):
                ins = args[:n_params]
                outs = args[n_params:]
                for _ in range(k):
                    outs = entry(*ins, *outs)
                return outs
            entry_k.__name__ = f"{entry.__name__}_x{k}"
            donate = tuple(range(n_params, n_params + len(self.out_names)))
            return jax.jit(entry_k, donate_argnums=donate, keep_unused=True)

        def stage(self, in_maps):
            import jax
            concat = [np.concatenate([in_maps[c][n] for c in range(N_CORES)], axis=0)
                      for n in self.in_names]
            return [jax.device_put(x, self.sharding) for x in concat]

        def zeros(self):
            import jax
            import jax.numpy as jnp
            if not hasattr(self, "_zeros_fn"):
                shapes = [((N_CORES * av.shape[0],) + tuple(av.shape[1:]), av.dtype)
                          for av in self.out_avals]
                self._zeros_fn = jax.jit(
                    lambda: tuple(jnp.zeros(s, d) for s, d in shapes),
                    out_shardings=tuple(self.sharding for _ in shapes))
            return list(self._zeros_fn())

        def run(self, dev_in, outs=None):
            if outs is None:
                outs = self.zeros()
            return self.fn(*dev_in, *outs)

        def split(self, outs):
            import jax
            jax.block_until_ready(outs)
            res = []
            for c in range(N_CORES):
                res.append({
                    n: np.asarray(outs[i]).reshape(
                        N_CORES, *self.out_avals[i].shape)[c]
                    for i, n in enumerate(self.out_names)})
            return res

    return Runner()


def get_runner():
    global _RUNNER
    if _RUNNER is None:
        _RUNNER = _make_runner()
    return _RUNNER


def kernel(hidden_states, cos, sin, w_qkv, w_o):
    r = get_runner()
    in_maps = shard_inputs(hidden_states, cos, sin, w_qkv, w_o)
    dev_in = r.stage(in_maps)
    outs = r.run(dev_in)
    results = r.split(outs)
    return gather_outputs(results)
